# revision 7
# baseline (speedup 1.0000x reference)
"""Trainium2 Bass kernel for nn_ASPPConv (gated dilated conv + BatchNorm + ReLU).

Reference computation (per batch element b):
    for k in 0..9:  out[b] += W[:,:,k] @ (x_shift_k[b] * g_k[b])
    g_k[b,n] = exp(-(|c0-ck|^2 + (d0-dk)^2)/2) * |cos(r0, rk)|
    out = relu(gamma * (out + bias - mean)/sqrt(var + eps) + beta)
  with mean/var batch statistics over (B, N) per channel.
  (bias cancels exactly inside train-mode BN, so it is skipped.)

Sharding: data-parallel over B=8 across 8 NeuronCores; per-channel BN
statistics are all-reduced across cores on device.

Per-core plan ([o,n] output orientation, contraction over c on partitions):
 - g is computed in a (tap,group)-blocked layout on 126 partitions in the
   log domain: g = exp(-0.5*((dc+dd) + ln(v*w + eps) - ln(u^2 + eps))),
   then round-tripped through DRAM as rows g_lin[9, n].
 - per 512-column tile: g rows are partition-broadcast via a stride-0 DMA,
   the scaled moving tensors t = x_shift * g are built in bf16 on DVE,
   and 36 bf16 matmuls (9 taps x 2 c-chunks x 2 o-chunks) accumulate in PSUM.
 - PSUM is evacuated on ScalarE with fused per-channel sum / sum-of-squares
   accumulation; stats are AllReduced; BN+ReLU applied as one ScalarE
   activation per tile with per-partition scale/bias.
"""

import numpy as np
import ml_dtypes

import concourse.bass as bass
import concourse.tile as tile
from concourse import bacc, mybir
from concourse.bass_utils import run_bass_kernel_spmd

NUM_CORES = 8
B, CIN, COUT, N = 8, 256, 256, 8192
K, DIL = 9, 6
PAD = DIL * (K // 2)          # 24
NXP = N + 2 * PAD             # 8240 padded x length
GRP, NGRP = 592, 14           # g blocked layout: 14 groups of 592
NG = GRP * NGRP               # 8288 blocked g domain (>= N)
NGP = NG + 2 * PAD            # 8336 padded coord length
PGK = K * NGRP                # 126 partitions for g phase
TS, NT = 512, 16              # main loop tiling
BN_EPS = 1e-5
G_EPS = 1e-30
INV_COUNT = 1.0 / (B * N)

F32 = mybir.dt.float32
BF16 = mybir.dt.bfloat16
AF = mybir.ActivationFunctionType
ALU = mybir.AluOpType

_CACHE = {}


def _build_kernel():
    nc = bacc.Bacc(
        "TRN2",
        target_bir_lowering=False,
        debug=False,
        enable_asserts=True,
        num_devices=NUM_CORES,
    )
    xp = nc.dram_tensor("xp", [2, 128, NXP], BF16, kind="ExternalInput").ap()
    cd4 = nc.dram_tensor("cd4", [4, NGP], F32, kind="ExternalInput").ap()
    rot = nc.dram_tensor("rot", [3, NGP], F32, kind="ExternalInput").ap()
    wt = nc.dram_tensor("wt", [36, 128, 128], BF16, kind="ExternalInput").ap()
    gam = nc.dram_tensor("gam", [2, 128], F32, kind="ExternalInput").ap()
    bet = nc.dram_tensor("bet", [2, 128], F32, kind="ExternalInput").ap()
    out = nc.dram_tensor("out", [COUT, N], F32, kind="ExternalOutput").ap()

    with tile.TileContext(nc) as tc:
        _body(tc, xp, cd4, rot, wt, gam, bet, out)
    nc.compile()
    return nc


def _body(tc, xp, cd4, rot, wt, gam, bet, out, dbg=None):
    nc = tc.nc
    from contextlib import ExitStack

    with ExitStack() as ctx:
        persist = ctx.enter_context(tc.tile_pool(name="persist", bufs=1))
        dram = ctx.enter_context(tc.tile_pool(name="dram", bufs=1, space="DRAM"))

        # ---- persistent SBUF tensors ----
        x_sb = persist.tile([128, 2, NXP], BF16)
        nc.sync.dma_start(out=x_sb[:], in_=bass.AP(
            tensor=xp.tensor, offset=0,
            ap=[[NXP, 128], [128 * NXP, 2], [1, NXP]],
        ))
        w_sb = persist.tile([128, 36, 128], BF16)
        nc.sync.dma_start(out=w_sb[:], in_=bass.AP(
            tensor=wt.tensor, offset=0,
            ap=[[128, 128], [128 * 128, 36], [1, 128]],
        ))
        gam_sb = persist.tile([128, 2], F32)
        nc.sync.dma_start(out=gam_sb[:], in_=bass.AP(
            tensor=gam.tensor, offset=0, ap=[[1, 128], [128, 2]]))
        bet_sb = persist.tile([128, 2], F32)
        nc.sync.dma_start(out=bet_sb[:], in_=bass.AP(
            tensor=bet.tensor, offset=0, ap=[[1, 128], [128, 2]]))

        conv_sb = persist.tile([128, 2, N], BF16)          # conv output (pre-BN)
        s1cols = persist.tile([128, 2, NT], F32)           # per-tile channel sums
        s2cols = persist.tile([128, 2, NT], F32)           # per-tile channel sumsq
        geps = persist.tile([PGK, 1], F32)
        nc.vector.memset(geps, G_EPS)
        bneps = persist.tile([128, 1], F32)
        nc.vector.memset(bneps, BN_EPS)

        g_lin = dram.tile([K, NG], BF16)

        # ---- phase 1: tap gate g in (tap,group)-blocked layout ----
        with tc.tile_pool(name="gphase", bufs=1) as gp:
            cd_s = gp.tile([PGK, 4, GRP], F32)    # shifted coords+dist
            cd_0 = gp.tile([PGK, 4, GRP], F32)    # unshifted, replicated per tap
            r_all = gp.tile([PGK, 2, 3, GRP], F32)  # [:,0]=unshifted, [:,1]=shifted
            for k in range(K):
                sl = slice(k * NGRP, (k + 1) * NGRP)
                nc.sync.dma_start(out=cd_s[sl], in_=bass.AP(
                    tensor=cd4.tensor, offset=k * DIL,
                    ap=[[GRP, NGRP], [NGP, 4], [1, GRP]]))
                nc.sync.dma_start(out=cd_0[sl], in_=bass.AP(
                    tensor=cd4.tensor, offset=PAD,
                    ap=[[GRP, NGRP], [NGP, 4], [1, GRP]]))
                nc.sync.dma_start(out=r_all[sl, 1], in_=bass.AP(
                    tensor=rot.tensor, offset=k * DIL,
                    ap=[[GRP, NGRP], [NGP, 3], [1, GRP]]))
                nc.sync.dma_start(out=r_all[sl, 0], in_=bass.AP(
                    tensor=rot.tensor, offset=PAD,
                    ap=[[GRP, NGRP], [NGP, 3], [1, GRP]]))

            diff = gp.tile([PGK, 4, GRP], F32)
            nc.vector.tensor_sub(diff[:], cd_s[:], cd_0[:])
            dsq = gp.tile([PGK, 4, GRP], F32)
            nc.scalar.activation(dsq[:], diff[:], AF.Square)
            q = gp.tile([PGK, GRP], F32)
            nc.vector.tensor_add(q[:], dsq[:, 0], dsq[:, 1])
            nc.vector.tensor_add(q[:], q[:], dsq[:, 2])
            nc.vector.tensor_add(q[:], q[:], dsq[:, 3])

            rt = gp.tile([PGK, 3, GRP], F32)
            nc.vector.tensor_mul(rt[:], r_all[:, 0], r_all[:, 1])
            u = gp.tile([PGK, GRP], F32)
            nc.vector.tensor_add(u[:], rt[:, 0], rt[:, 1])
            nc.vector.tensor_add(u[:], u[:], rt[:, 2])
            usq = gp.tile([PGK, GRP], F32)
            nc.scalar.activation(usq[:], u[:], AF.Square)

            rsq = gp.tile([PGK, 2, 3, GRP], F32)
            nc.scalar.activation(rsq[:], r_all[:], AF.Square)
            v = gp.tile([PGK, GRP], F32)
            nc.vector.tensor_add(v[:], rsq[:, 0, 0], rsq[:, 0, 1])
            nc.vector.tensor_add(v[:], v[:], rsq[:, 0, 2])
            w_ = gp.tile([PGK, GRP], F32)
            nc.vector.tensor_add(w_[:], rsq[:, 1, 0], rsq[:, 1, 1])
            nc.vector.tensor_add(w_[:], w_[:], rsq[:, 1, 2])
            vw = gp.tile([PGK, GRP], F32)
            nc.vector.tensor_mul(vw[:], v[:], w_[:])

            lvw = gp.tile([PGK, GRP], F32)
            nc.scalar.activation(lvw[:], vw[:], AF.Ln, bias=geps[:])
            lu = gp.tile([PGK, GRP], F32)
            nc.scalar.activation(lu[:], usq[:], AF.Ln, bias=geps[:])
            nc.vector.tensor_add(q[:], q[:], lvw[:])
            nc.vector.tensor_sub(q[:], q[:], lu[:])
            gblk = gp.tile([PGK, GRP], BF16)
            nc.scalar.activation(gblk[:], q[:], AF.Exp, scale=-0.5)

            # blocked [126, 592] -> linear rows g_lin[9, 8288] in DRAM
            nc.sync.dma_start(out=bass.AP(
                tensor=g_lin.tensor, offset=g_lin.offset,
                ap=[[NG, K], [GRP, NGRP], [1, GRP]],
            ), in_=gblk[:])
            if dbg is not None:
                nc.gpsimd.dma_start(out=bass.AP(
                    tensor=dbg["g"].tensor, offset=0,
                    ap=[[NG, K], [GRP, NGRP], [1, GRP]],
                ), in_=gblk[:])

        # ---- phase 2: main conv loop over 16 tiles of 512 ----
        gb_pool = ctx.enter_context(tc.tile_pool(name="gb", bufs=3))
        tm_pool = ctx.enter_context(tc.tile_pool(name="tmul", bufs=36))
        ps_pool = ctx.enter_context(tc.tile_pool(name="psum", bufs=4, space="PSUM"))
        sq_pool = ctx.enter_context(tc.tile_pool(name="sqs", bufs=2))

        for t in range(NT):
            gb_t = gb_pool.tile([128, K, TS], BF16, tag="gb")
            nc.sync.dma_start(out=gb_t[:], in_=bass.AP(
                tensor=g_lin.tensor, offset=g_lin.offset + t * TS,
                ap=[[0, 128], [NG, K], [1, TS]],
            ))
            tmul = [[None] * 2 for _ in range(K)]
            for k in range(K):
                for cc in range(2):
                    tm = tm_pool.tile([128, TS], BF16, tag="tm")
                    nc.vector.tensor_mul(
                        tm[:],
                        x_sb[:, cc, k * DIL + t * TS: k * DIL + t * TS + TS],
                        gb_t[:, k, :],
                    )
                    tmul[k][cc] = tm
            for oc in range(2):
                ps = ps_pool.tile([128, TS], F32, tag="ps")
                idx = 0
                for cc in range(2):
                    for k in range(K):
                        nc.tensor.matmul(
                            ps[:],
                            w_sb[:, k * 4 + cc * 2 + oc, :],
                            tmul[k][cc][:],
                            start=(idx == 0),
                            stop=(idx == 17),
                        )
                        idx += 1
                nc.scalar.activation(
                    conv_sb[:, oc, t * TS: (t + 1) * TS], ps[:], AF.Copy,
                    accum_out=s1cols[:, oc, t: t + 1],
                )
                sq = sq_pool.tile([128, TS], BF16, tag="sq")
                nc.scalar.activation(
                    sq[:], ps[:], AF.Square,
                    accum_out=s2cols[:, oc, t: t + 1],
                )

        if dbg is not None:
            nc.gpsimd.dma_start(out=bass.AP(
                tensor=dbg["conv"].tensor, offset=0,
                ap=[[N, 128], [128 * N, 2], [1, N]],
            ), in_=conv_sb[:])

        # ---- phase 3: stats all-reduce + BN coefficients ----
        stats = persist.tile([128, 4], F32)
        nc.vector.tensor_reduce(stats[:, 0:2], s1cols[:], axis=mybir.AxisListType.X,
                                op=ALU.add)
        nc.vector.tensor_reduce(stats[:, 2:4], s2cols[:], axis=mybir.AxisListType.X,
                                op=ALU.add)
        cc_in = dram.tile([128, 4], F32)
        cc_out = dram.tile([128, 4], F32)
        nc.sync.dma_start(out=cc_in[:], in_=stats[:])
        nc.gpsimd.collective_compute(
            "AllReduce", ALU.add,
            replica_groups=[list(range(NUM_CORES))],
            ins=[cc_in.opt()], outs=[cc_out.opt()],
        )
        red = persist.tile([128, 4], F32)
        nc.sync.dma_start(out=red[:], in_=cc_out[:])
        if dbg is not None:
            nc.sync.dma_start(out=dbg["stats"][:], in_=stats[:])
            nc.sync.dma_start(out=dbg["red"][:], in_=red[:])

        m = persist.tile([128, 2], F32)
        nc.vector.tensor_scalar_mul(m[:], red[:, 0:2], INV_COUNT)
        e2 = persist.tile([128, 2], F32)
        nc.vector.tensor_scalar_mul(e2[:], red[:, 2:4], INV_COUNT)
        var = persist.tile([128, 2], F32)
        nc.vector.tensor_mul(var[:], m[:], m[:])
        nc.vector.tensor_sub(var[:], e2[:], var[:])
        # rinv = 1/sqrt(var+eps) = exp(-0.5*ln(var+eps))
        lv = persist.tile([128, 2], F32)
        nc.scalar.activation(lv[:], var[:], AF.Ln, bias=bneps[:])
        rinv = persist.tile([128, 2], F32)
        nc.scalar.activation(rinv[:], lv[:], AF.Exp, scale=-0.5)
        scl = persist.tile([128, 2], F32)
        nc.vector.tensor_mul(scl[:], rinv[:], gam_sb[:])
        bia = persist.tile([128, 2], F32)
        nc.vector.tensor_mul(bia[:], m[:], scl[:])
        nc.vector.tensor_sub(bia[:], bet_sb[:], bia[:])

        # ---- phase 4: BN + ReLU + store ----
        fin_pool = ctx.enter_context(tc.tile_pool(name="fin", bufs=4))
        for t in range(NT):
            for oc in range(2):
                fin = fin_pool.tile([128, TS], F32, tag="fin")
                nc.scalar.activation(
                    fin[:], conv_sb[:, oc, t * TS: (t + 1) * TS], AF.Relu,
                    bias=bia[:, oc: oc + 1], scale=scl[:, oc: oc + 1],
                )
                nc.sync.dma_start(
                    out=out[oc * 128: (oc + 1) * 128, t * TS: (t + 1) * TS],
                    in_=fin[:],
                )


def _prep_inputs(x, coords, rotations, distances, W, gamma, beta):
    """Host-side sharding/layout prep. Returns per-core input maps."""
    bf = ml_dtypes.bfloat16
    # weights: [o, c, k] -> 36 lhsT tiles [(k, cc, oc), c, o]
    wt = W.reshape(2, 128, 2, 128, K)            # [oc, o, cc, c, k]
    wt = wt.transpose(4, 2, 0, 3, 1)             # [k, cc, oc, c, o]
    wt = np.ascontiguousarray(wt.reshape(36, 128, 128), dtype=bf)
    gam2 = np.ascontiguousarray(gamma.reshape(2, 128), dtype=np.float32)
    bet2 = np.ascontiguousarray(beta.reshape(2, 128), dtype=np.float32)

    in_maps = []
    for b in range(NUM_CORES):
        xpad = np.zeros((CIN, NXP), dtype=bf)
        xpad[:, PAD: PAD + N] = x[b].astype(bf)
        cd4 = np.zeros((4, NGP), dtype=np.float32)
        cd4[:3, PAD: PAD + N] = coords[b]
        cd4[3, PAD: PAD + N] = distances[b]
        rot = np.zeros((3, NGP), dtype=np.float32)
        rot[:, PAD: PAD + N] = rotations[b]
        in_maps.append({
            "xp": np.ascontiguousarray(xpad.reshape(2, 128, NXP)),
            "cd4": cd4,
            "rot": rot,
            "wt": wt,
            "gam": gam2,
            "bet": bet2,
        })
    return in_maps


def kernel(x, coords, rotations, distances, W, bias, gamma, beta):
    if "nc" not in _CACHE:
        _CACHE["nc"] = _build_kernel()
    nc = _CACHE["nc"]
    in_maps = _prep_inputs(x, coords, rotations, distances, W, gamma, beta)
    res = run_bass_kernel_spmd(nc, in_maps, list(range(NUM_CORES)), trace=False)
    return np.stack([res.results[b]["out"] for b in range(NUM_CORES)], axis=0)


# revision 8
# speedup vs baseline: 13.7061x; 13.7061x over previous
"""Trainium2 Bass kernel for nn_ASPPConv (gated dilated conv + BatchNorm + ReLU).

Reference computation (per batch element b):
    for k in 0..9:  out[b] += W[:,:,k] @ (x_shift_k[b] * g_k[b])
    g_k[b,n] = exp(-(|c0-ck|^2 + (d0-dk)^2)/2) * |cos(r0, rk)|
    out = relu(gamma * (out + bias - mean)/sqrt(var + eps) + beta)
  with mean/var batch statistics over (B, N) per channel.
  (bias cancels exactly inside train-mode BN, so it is skipped.)

Sharding: data-parallel over B=8 across 8 NeuronCores; per-channel BN
statistics are all-reduced across cores on device.

Per-core plan ([o,n] output orientation, contraction over c on partitions):
 - g is computed in a (tap,group)-blocked layout on 126 partitions in the
   log domain: g = exp(-0.5*((dc+dd) + ln(v*w + eps) - ln(u^2 + eps))),
   then round-tripped through DRAM as rows g_lin[9, n].
 - per 512-column tile: g rows are partition-broadcast via a stride-0 DMA,
   the scaled moving tensors t = x_shift * g are built in bf16 on DVE,
   and 36 bf16 matmuls (9 taps x 2 c-chunks x 2 o-chunks) accumulate in PSUM.
 - PSUM is evacuated on ScalarE with fused per-channel sum / sum-of-squares
   accumulation; stats are AllReduced; BN+ReLU applied as one ScalarE
   activation per tile with per-partition scale/bias.
"""

import numpy as np
import ml_dtypes

import concourse.bass as bass
import concourse.tile as tile
from concourse import bacc, mybir
from concourse.bass_utils import run_bass_kernel_spmd

NUM_CORES = 8
B, CIN, COUT, N = 8, 256, 256, 8192
K, DIL = 9, 6
PAD = DIL * (K // 2)          # 24
NXP = N + 2 * PAD             # 8240 padded x length
GRP, NGRP = 592, 14           # g blocked layout: 14 groups of 592
NG = GRP * NGRP               # 8288 blocked g domain (>= N)
NGP = NG + 2 * PAD            # 8336 padded coord length
PGK = K * NGRP                # 126 partitions for g phase
TS, NT = 512, 16              # main loop tiling
BN_EPS = 1e-5
G_EPS = 1e-30
INV_COUNT = 1.0 / (B * N)

F32 = mybir.dt.float32
BF16 = mybir.dt.bfloat16
AF = mybir.ActivationFunctionType
ALU = mybir.AluOpType

_CACHE = {}


def _build_kernel(reps=1):
    nc = bacc.Bacc(
        "TRN2",
        target_bir_lowering=False,
        debug=False,
        enable_asserts=True,
        num_devices=NUM_CORES,
    )
    xp = nc.dram_tensor("xp", [2, 128, NXP], BF16, kind="ExternalInput").ap()
    cd4 = nc.dram_tensor("cd4", [4, NGP], F32, kind="ExternalInput").ap()
    rot = nc.dram_tensor("rot", [3, NGP], F32, kind="ExternalInput").ap()
    wt = nc.dram_tensor("wt", [36, 128, 128], BF16, kind="ExternalInput").ap()
    gam = nc.dram_tensor("gam", [2, 128], F32, kind="ExternalInput").ap()
    bet = nc.dram_tensor("bet", [2, 128], F32, kind="ExternalInput").ap()
    out = nc.dram_tensor("out", [COUT, N], F32, kind="ExternalOutput").ap()

    with tile.TileContext(nc) as tc:
        for r in range(reps):
            _body(tc, xp, cd4, rot, wt, gam, bet, out, uid=str(r))
    nc.compile()
    return nc


def _body(tc, xp, cd4, rot, wt, gam, bet, out, dbg=None, uid=""):
    nc = tc.nc
    from contextlib import ExitStack

    with ExitStack() as ctx:
        persist = ctx.enter_context(tc.tile_pool(name=f"persist{uid}", bufs=1))
        dram = ctx.enter_context(tc.tile_pool(name=f"dram{uid}", bufs=1, space="DRAM"))

        # ---- persistent SBUF tensors ----
        x_sb = persist.tile([128, 2, NXP], BF16)
        nc.sync.dma_start(out=x_sb[:], in_=bass.AP(
            tensor=xp.tensor, offset=0,
            ap=[[NXP, 128], [128 * NXP, 2], [1, NXP]],
        ))
        w_sb = persist.tile([128, 36, 128], BF16)
        nc.sync.dma_start(out=w_sb[:], in_=bass.AP(
            tensor=wt.tensor, offset=0,
            ap=[[128, 128], [128 * 128, 36], [1, 128]],
        ))
        gam_sb = persist.tile([128, 2], F32)
        nc.sync.dma_start(out=gam_sb[:], in_=bass.AP(
            tensor=gam.tensor, offset=0, ap=[[1, 128], [128, 2]]))
        bet_sb = persist.tile([128, 2], F32)
        nc.sync.dma_start(out=bet_sb[:], in_=bass.AP(
            tensor=bet.tensor, offset=0, ap=[[1, 128], [128, 2]]))

        conv_sb = persist.tile([128, 2, N], BF16)          # conv output (pre-BN)
        s1cols = persist.tile([128, 2, NT], F32)           # per-tile channel sums
        s2cols = persist.tile([128, 2, NT], F32)           # per-tile channel sumsq
        geps = persist.tile([PGK, 1], F32)
        nc.vector.memset(geps, G_EPS)
        bneps = persist.tile([128, 1], F32)
        nc.vector.memset(bneps, BN_EPS)

        g_lin = dram.tile([K, NG], BF16)

        # ---- phase 1: tap gate g in (tap,group)-blocked layout ----
        with tc.tile_pool(name=f"gphase{uid}", bufs=1) as gp:
            cd_s = gp.tile([PGK, 4, GRP], F32)    # shifted coords+dist
            cd_0 = gp.tile([PGK, 4, GRP], F32)    # unshifted, replicated per tap
            r_all = gp.tile([PGK, 2, 3, GRP], F32)  # [:,0]=unshifted, [:,1]=shifted
            for k in range(K):
                sl = slice(k * NGRP, (k + 1) * NGRP)
                nc.sync.dma_start(out=cd_s[sl], in_=bass.AP(
                    tensor=cd4.tensor, offset=k * DIL,
                    ap=[[GRP, NGRP], [NGP, 4], [1, GRP]]))
                nc.sync.dma_start(out=cd_0[sl], in_=bass.AP(
                    tensor=cd4.tensor, offset=PAD,
                    ap=[[GRP, NGRP], [NGP, 4], [1, GRP]]))
                nc.sync.dma_start(out=r_all[sl, 1], in_=bass.AP(
                    tensor=rot.tensor, offset=k * DIL,
                    ap=[[GRP, NGRP], [NGP, 3], [1, GRP]]))
                nc.sync.dma_start(out=r_all[sl, 0], in_=bass.AP(
                    tensor=rot.tensor, offset=PAD,
                    ap=[[GRP, NGRP], [NGP, 3], [1, GRP]]))

            diff = gp.tile([PGK, 4, GRP], F32)
            nc.vector.tensor_sub(diff[:], cd_s[:], cd_0[:])
            dsq = gp.tile([PGK, 4, GRP], F32)
            nc.scalar.activation(dsq[:], diff[:], AF.Square)
            q = gp.tile([PGK, GRP], F32)
            nc.vector.tensor_add(q[:], dsq[:, 0], dsq[:, 1])
            nc.vector.tensor_add(q[:], q[:], dsq[:, 2])
            nc.vector.tensor_add(q[:], q[:], dsq[:, 3])

            rt = gp.tile([PGK, 3, GRP], F32)
            nc.vector.tensor_mul(rt[:], r_all[:, 0], r_all[:, 1])
            u = gp.tile([PGK, GRP], F32)
            nc.vector.tensor_add(u[:], rt[:, 0], rt[:, 1])
            nc.vector.tensor_add(u[:], u[:], rt[:, 2])
            usq = gp.tile([PGK, GRP], F32)
            nc.scalar.activation(usq[:], u[:], AF.Square)

            rsq = gp.tile([PGK, 2, 3, GRP], F32)
            nc.scalar.activation(rsq[:], r_all[:], AF.Square)
            v = gp.tile([PGK, GRP], F32)
            nc.vector.tensor_add(v[:], rsq[:, 0, 0], rsq[:, 0, 1])
            nc.vector.tensor_add(v[:], v[:], rsq[:, 0, 2])
            w_ = gp.tile([PGK, GRP], F32)
            nc.vector.tensor_add(w_[:], rsq[:, 1, 0], rsq[:, 1, 1])
            nc.vector.tensor_add(w_[:], w_[:], rsq[:, 1, 2])
            vw = gp.tile([PGK, GRP], F32)
            nc.vector.tensor_mul(vw[:], v[:], w_[:])

            lvw = gp.tile([PGK, GRP], F32)
            nc.scalar.activation(lvw[:], vw[:], AF.Ln, bias=geps[:])
            lu = gp.tile([PGK, GRP], F32)
            nc.scalar.activation(lu[:], usq[:], AF.Ln, bias=geps[:])
            nc.vector.tensor_add(q[:], q[:], lvw[:])
            nc.vector.tensor_sub(q[:], q[:], lu[:])
            gblk = gp.tile([PGK, GRP], BF16)
            nc.scalar.activation(gblk[:], q[:], AF.Exp, scale=-0.5)

            # blocked [126, 592] -> linear rows g_lin[9, 8288] in DRAM
            nc.sync.dma_start(out=bass.AP(
                tensor=g_lin.tensor, offset=g_lin.offset,
                ap=[[NG, K], [GRP, NGRP], [1, GRP]],
            ), in_=gblk[:])
            if dbg is not None:
                nc.gpsimd.dma_start(out=bass.AP(
                    tensor=dbg["g"].tensor, offset=0,
                    ap=[[NG, K], [GRP, NGRP], [1, GRP]],
                ), in_=gblk[:])

        # ---- phase 2: main conv loop over 16 tiles of 512 ----
        gb_pool = ctx.enter_context(tc.tile_pool(name=f"gb{uid}", bufs=3))
        tm_pool = ctx.enter_context(tc.tile_pool(name=f"tmul{uid}", bufs=36))
        ps_pool = ctx.enter_context(tc.tile_pool(name=f"psum{uid}", bufs=4, space="PSUM"))
        sq_pool = ctx.enter_context(tc.tile_pool(name=f"sqs{uid}", bufs=2))

        for t in range(NT):
            gb_t = gb_pool.tile([128, K, TS], BF16, tag="gb")
            nc.sync.dma_start(out=gb_t[:], in_=bass.AP(
                tensor=g_lin.tensor, offset=g_lin.offset + t * TS,
                ap=[[0, 128], [NG, K], [1, TS]],
            ))
            tmul = [[None] * 2 for _ in range(K)]
            for k in range(K):
                for cc in range(2):
                    tm = tm_pool.tile([128, TS], BF16, tag="tm")
                    nc.vector.tensor_mul(
                        tm[:],
                        x_sb[:, cc, k * DIL + t * TS: k * DIL + t * TS + TS],
                        gb_t[:, k, :],
                    )
                    tmul[k][cc] = tm
            for oc in range(2):
                ps = ps_pool.tile([128, TS], F32, tag="ps")
                idx = 0
                for cc in range(2):
                    for k in range(K):
                        nc.tensor.matmul(
                            ps[:],
                            w_sb[:, k * 4 + cc * 2 + oc, :],
                            tmul[k][cc][:],
                            start=(idx == 0),
                            stop=(idx == 17),
                        )
                        idx += 1
                nc.scalar.activation(
                    conv_sb[:, oc, t * TS: (t + 1) * TS], ps[:], AF.Copy,
                    accum_out=s1cols[:, oc, t: t + 1],
                )
                sq = sq_pool.tile([128, TS], BF16, tag="sq")
                nc.scalar.activation(
                    sq[:], ps[:], AF.Square,
                    accum_out=s2cols[:, oc, t: t + 1],
                )

        if dbg is not None:
            nc.gpsimd.dma_start(out=bass.AP(
                tensor=dbg["conv"].tensor, offset=0,
                ap=[[N, 128], [128 * N, 2], [1, N]],
            ), in_=conv_sb[:])

        # ---- phase 3: stats all-reduce + BN coefficients ----
        stats = persist.tile([128, 4], F32)
        nc.vector.tensor_reduce(stats[:, 0:2], s1cols[:], axis=mybir.AxisListType.X,
                                op=ALU.add)
        nc.vector.tensor_reduce(stats[:, 2:4], s2cols[:], axis=mybir.AxisListType.X,
                                op=ALU.add)
        cc_in = dram.tile([128, 4], F32)
        cc_out = dram.tile([128, 4], F32)
        nc.sync.dma_start(out=cc_in[:], in_=stats[:])
        nc.gpsimd.collective_compute(
            "AllReduce", ALU.add,
            replica_groups=[list(range(NUM_CORES))],
            ins=[cc_in.opt()], outs=[cc_out.opt()],
        )
        red = persist.tile([128, 4], F32)
        nc.sync.dma_start(out=red[:], in_=cc_out[:])
        if dbg is not None:
            nc.sync.dma_start(out=dbg["stats"][:], in_=stats[:])
            nc.sync.dma_start(out=dbg["red"][:], in_=red[:])

        m = persist.tile([128, 2], F32)
        nc.vector.tensor_scalar_mul(m[:], red[:, 0:2], INV_COUNT)
        e2 = persist.tile([128, 2], F32)
        nc.vector.tensor_scalar_mul(e2[:], red[:, 2:4], INV_COUNT)
        var = persist.tile([128, 2], F32)
        nc.vector.tensor_mul(var[:], m[:], m[:])
        nc.vector.tensor_sub(var[:], e2[:], var[:])
        # rinv = 1/sqrt(var+eps) = exp(-0.5*ln(var+eps))
        lv = persist.tile([128, 2], F32)
        nc.scalar.activation(lv[:], var[:], AF.Ln, bias=bneps[:])
        rinv = persist.tile([128, 2], F32)
        nc.scalar.activation(rinv[:], lv[:], AF.Exp, scale=-0.5)
        scl = persist.tile([128, 2], F32)
        nc.vector.tensor_mul(scl[:], rinv[:], gam_sb[:])
        bia = persist.tile([128, 2], F32)
        nc.vector.tensor_mul(bia[:], m[:], scl[:])
        nc.vector.tensor_sub(bia[:], bet_sb[:], bia[:])

        # ---- phase 4: BN + ReLU + store ----
        fin_pool = ctx.enter_context(tc.tile_pool(name=f"fin{uid}", bufs=4))
        for t in range(NT):
            for oc in range(2):
                fin = fin_pool.tile([128, TS], F32, tag="fin")
                nc.scalar.activation(
                    fin[:], conv_sb[:, oc, t * TS: (t + 1) * TS], AF.Relu,
                    bias=bia[:, oc: oc + 1], scale=scl[:, oc: oc + 1],
                )
                nc.sync.dma_start(
                    out=out[oc * 128: (oc + 1) * 128, t * TS: (t + 1) * TS],
                    in_=fin[:],
                )


def _prep_inputs(x, coords, rotations, distances, W, gamma, beta):
    """Host-side sharding/layout prep. Returns per-core input maps."""
    bf = ml_dtypes.bfloat16
    # weights: [o, c, k] -> 36 lhsT tiles [(k, cc, oc), c, o]
    wt = W.reshape(2, 128, 2, 128, K)            # [oc, o, cc, c, k]
    wt = wt.transpose(4, 2, 0, 3, 1)             # [k, cc, oc, c, o]
    wt = np.ascontiguousarray(wt.reshape(36, 128, 128), dtype=bf)
    gam2 = np.ascontiguousarray(gamma.reshape(2, 128), dtype=np.float32)
    bet2 = np.ascontiguousarray(beta.reshape(2, 128), dtype=np.float32)

    in_maps = []
    for b in range(NUM_CORES):
        xpad = np.zeros((CIN, NXP), dtype=bf)
        xpad[:, PAD: PAD + N] = x[b].astype(bf)
        cd4 = np.zeros((4, NGP), dtype=np.float32)
        cd4[:3, PAD: PAD + N] = coords[b]
        cd4[3, PAD: PAD + N] = distances[b]
        rot = np.zeros((3, NGP), dtype=np.float32)
        rot[:, PAD: PAD + N] = rotations[b]
        in_maps.append({
            "xp": np.ascontiguousarray(xpad.reshape(2, 128, NXP)),
            "cd4": cd4,
            "rot": rot,
            "wt": wt,
            "gam": gam2,
            "bet": bet2,
        })
    return in_maps


def kernel(x, coords, rotations, distances, W, bias, gamma, beta):
    if "nc" not in _CACHE:
        _CACHE["nc"] = _build_kernel()
    nc = _CACHE["nc"]
    in_maps = _prep_inputs(x, coords, rotations, distances, W, gamma, beta)
    res = run_bass_kernel_spmd(nc, in_maps, list(range(NUM_CORES)), trace=False)
    return np.stack([res.results[b]["out"] for b in range(NUM_CORES)], axis=0)


# revision 18
# speedup vs baseline: 20.5452x; 1.4990x over previous
"""Trainium2 Bass kernel for nn_ASPPConv (gated dilated conv + BatchNorm + ReLU).

Reference computation (per batch element b):
    for k in 0..9:  out[b] += W[:,:,k] @ (x_shift_k[b] * g_k[b])
    g_k[b,n] = exp(-(|c0-ck|^2 + (d0-dk)^2)/2) * |cos(r0, rk)|
    out = relu(gamma * (out + bias - mean)/sqrt(var + eps) + beta)
  with mean/var batch statistics over (B, N) per channel.
  (bias cancels exactly inside train-mode BN, so it is skipped.)

Sharding: data-parallel over B=8 across 8 NeuronCores; per-channel BN
statistics are all-reduced across cores on device.

Per-core plan ([o,n] output orientation, contraction over c on partitions):
 - g is computed in a (tap,group)-blocked layout on 126 partitions in the
   log domain: g = exp(-0.5*((dc+dd) + ln(v*w + eps) - ln(u^2 + eps))),
   then round-tripped through DRAM as rows g_lin[9, n].
 - per 512-column tile: g rows are partition-broadcast via a stride-0 DMA,
   the scaled moving tensors t = x_shift * g are built in bf16 on DVE,
   and 36 bf16 matmuls (9 taps x 2 c-chunks x 2 o-chunks) accumulate in PSUM.
 - PSUM is evacuated on ScalarE with fused per-channel sum / sum-of-squares
   accumulation; stats are AllReduced; BN+ReLU applied as one ScalarE
   activation per tile with per-partition scale/bias.
"""

import numpy as np
import ml_dtypes

import concourse.bass as bass
import concourse.tile as tile
from concourse import bacc, mybir
from concourse.bass_utils import run_bass_kernel_spmd

NUM_CORES = 8
B, CIN, COUT, N = 8, 256, 256, 8192
K, DIL = 9, 6
PAD = DIL * (K // 2)          # 24
NXP = N + 2 * PAD             # 8240 padded x length
GRP, NGRP = 592, 14           # g blocked layout: 14 groups of 592
NG = GRP * NGRP               # 8288 blocked g domain (>= N)
NGP = NG + 2 * PAD            # 8336 padded coord length
PGK = K * NGRP                # 126 partitions for g phase
TS, NT = 512, 16              # main loop tiling
BN_EPS = 1e-5
G_EPS = 1e-30
INV_COUNT = 1.0 / (B * N)

F32 = mybir.dt.float32
BF16 = mybir.dt.bfloat16
AF = mybir.ActivationFunctionType
ALU = mybir.AluOpType

_CACHE = {}


def _build_kernel(reps=1):
    nc = bacc.Bacc(
        "TRN2",
        target_bir_lowering=False,
        debug=False,
        enable_asserts=True,
        num_devices=NUM_CORES,
    )
    xp = nc.dram_tensor("xp", [2, 128, NXP], BF16, kind="ExternalInput").ap()
    cds = nc.dram_tensor("cds", [PGK, 4, GRP], F32, kind="ExternalInput").ap()
    cd0 = nc.dram_tensor("cd0", [PGK, 4, GRP], F32, kind="ExternalInput").ap()
    ral = nc.dram_tensor("ral", [PGK, 2, 3, GRP], F32, kind="ExternalInput").ap()
    wt = nc.dram_tensor("wt", [36, 128, 128], BF16, kind="ExternalInput").ap()
    gam = nc.dram_tensor("gam", [2, 128], F32, kind="ExternalInput").ap()
    bet = nc.dram_tensor("bet", [2, 128], F32, kind="ExternalInput").ap()
    out = nc.dram_tensor("out", [COUT, N], F32, kind="ExternalOutput").ap()

    with tile.TileContext(nc) as tc:
        for r in range(reps):
            _body(tc, xp, cds, cd0, ral, wt, gam, bet, out, uid=str(r))
    nc.compile()
    return nc


def _body(tc, xp, cds, cd0, ral, wt, gam, bet, out, dbg=None, uid="", single=False):
    nc = tc.nc
    from contextlib import ExitStack

    with ExitStack() as ctx:
        persist = ctx.enter_context(tc.tile_pool(name=f"persist{uid}", bufs=1))
        dram = ctx.enter_context(tc.tile_pool(name=f"dram{uid}", bufs=1, space="DRAM"))

        conv_sb = persist.tile([128, 2, N], BF16)          # conv output (pre-BN)
        s1cols = persist.tile([128, 2, NT], F32)           # per-tile channel sums
        s2cols = persist.tile([128, 2, NT], F32)           # per-tile channel sumsq
        geps = persist.tile([PGK, 1], F32)
        nc.vector.memset(geps, G_EPS)
        bneps = persist.tile([128, 1], F32)
        nc.vector.memset(bneps, BN_EPS)

        g_lin = dram.tile([K, NG], BF16)

        # ---- phase 1: tap gate g in (tap,group)-blocked layout ----
        # g-input DMAs are issued FIRST (they gate the g chain that gates the
        # whole main loop); x/W loads follow and overlap with g compute.
        with tc.tile_pool(name=f"gphase{uid}", bufs=1) as gp:
            cd_s = gp.tile([PGK, 4, GRP], F32)    # shifted coords+dist
            cd_0 = gp.tile([PGK, 4, GRP], F32)    # unshifted, replicated per tap
            r_all = gp.tile([PGK, 2, 3, GRP], F32)  # [:,0]=unshifted, [:,1]=shifted
            # host pre-gathered blocked views: contiguous loads
            nc.sync.dma_start(out=cd_s[:], in_=cds[:])
            nc.sync.dma_start(out=cd_0[:], in_=cd0[:])
            nc.sync.dma_start(out=r_all[:], in_=ral[:])

            # persistent loads (issued after g inputs; overlap g compute)
            x_sb = persist.tile([128, 2, NXP], BF16)
            nc.sync.dma_start(out=x_sb[:], in_=bass.AP(
                tensor=xp.tensor, offset=0,
                ap=[[NXP, 128], [128 * NXP, 2], [1, NXP]],
            ))
            w_sb = persist.tile([128, 36, 128], BF16)
            nc.sync.dma_start(out=w_sb[:], in_=bass.AP(
                tensor=wt.tensor, offset=0,
                ap=[[128, 128], [128 * 128, 36], [1, 128]],
            ))
            gam_sb = persist.tile([128, 2], F32)
            nc.sync.dma_start(out=gam_sb[:], in_=bass.AP(
                tensor=gam.tensor, offset=0, ap=[[1, 128], [128, 2]]))
            bet_sb = persist.tile([128, 2], F32)
            nc.sync.dma_start(out=bet_sb[:], in_=bass.AP(
                tensor=bet.tensor, offset=0, ap=[[1, 128], [128, 2]]))

            diff = gp.tile([PGK, 4, GRP], F32)
            nc.vector.tensor_sub(diff[:], cd_s[:], cd_0[:])
            dsq = gp.tile([PGK, 4, GRP], F32)
            nc.scalar.activation(dsq[:], diff[:], AF.Square)
            q = gp.tile([PGK, GRP], F32)
            nc.vector.tensor_add(q[:], dsq[:, 0], dsq[:, 1])
            nc.vector.tensor_add(q[:], q[:], dsq[:, 2])
            nc.vector.tensor_add(q[:], q[:], dsq[:, 3])

            rt = gp.tile([PGK, 3, GRP], F32)
            nc.vector.tensor_mul(rt[:], r_all[:, 0], r_all[:, 1])
            u = gp.tile([PGK, GRP], F32)
            nc.vector.tensor_add(u[:], rt[:, 0], rt[:, 1])
            nc.vector.tensor_add(u[:], u[:], rt[:, 2])
            usq = gp.tile([PGK, GRP], F32)
            nc.scalar.activation(usq[:], u[:], AF.Square)

            rsq = gp.tile([PGK, 2, 3, GRP], F32)
            nc.scalar.activation(rsq[:], r_all[:], AF.Square)
            v = gp.tile([PGK, GRP], F32)
            nc.vector.tensor_add(v[:], rsq[:, 0, 0], rsq[:, 0, 1])
            nc.vector.tensor_add(v[:], v[:], rsq[:, 0, 2])
            w_ = gp.tile([PGK, GRP], F32)
            nc.vector.tensor_add(w_[:], rsq[:, 1, 0], rsq[:, 1, 1])
            nc.vector.tensor_add(w_[:], w_[:], rsq[:, 1, 2])
            vw = gp.tile([PGK, GRP], F32)
            nc.vector.tensor_mul(vw[:], v[:], w_[:])

            lvw = gp.tile([PGK, GRP], F32)
            nc.scalar.activation(lvw[:], vw[:], AF.Ln, bias=geps[:])
            lu = gp.tile([PGK, GRP], F32)
            nc.scalar.activation(lu[:], usq[:], AF.Ln, bias=geps[:])
            nc.vector.tensor_add(q[:], q[:], lvw[:])
            nc.vector.tensor_sub(q[:], q[:], lu[:])
            gblk = gp.tile([PGK, GRP], BF16)
            nc.scalar.activation(gblk[:], q[:], AF.Exp, scale=-0.5)

            # blocked [126, 592] -> linear rows g_lin[9, 8288] in DRAM
            nc.sync.dma_start(out=bass.AP(
                tensor=g_lin.tensor, offset=g_lin.offset,
                ap=[[NG, K], [GRP, NGRP], [1, GRP]],
            ), in_=gblk[:])
            if dbg is not None:
                nc.gpsimd.dma_start(out=bass.AP(
                    tensor=dbg["g"].tensor, offset=0,
                    ap=[[NG, K], [GRP, NGRP], [1, GRP]],
                ), in_=gblk[:])

        # ---- phase 2: main conv loop over 16 tiles of 512 ----
        gb_pool = ctx.enter_context(tc.tile_pool(name=f"gb{uid}", bufs=4))
        tm_pool = ctx.enter_context(tc.tile_pool(name=f"tmul{uid}", bufs=54))
        ps_pool = ctx.enter_context(tc.tile_pool(name=f"psum{uid}", bufs=6, space="PSUM"))
        sq_pool = ctx.enter_context(tc.tile_pool(name=f"sqs{uid}", bufs=2))

        for t in range(NT):
            gb_t = gb_pool.tile([128, K, TS], BF16, tag="gb")
            nc.sync.dma_start(out=gb_t[:], in_=bass.AP(
                tensor=g_lin.tensor, offset=g_lin.offset + t * TS,
                ap=[[0, 128], [NG, K], [1, TS]],
            ))
            tmul = [[None] * 2 for _ in range(K)]
            for k in range(K):
                for cc in range(2):
                    tm = tm_pool.tile([128, TS], BF16, tag="tm")
                    nc.vector.tensor_mul(
                        tm[:],
                        x_sb[:, cc, k * DIL + t * TS: k * DIL + t * TS + TS],
                        gb_t[:, k, :],
                    )
                    tmul[k][cc] = tm
            for oc in range(2):
                ps = ps_pool.tile([128, TS], F32, tag="ps")
                idx = 0
                for cc in range(2):
                    for k in range(K):
                        nc.tensor.matmul(
                            ps[:],
                            w_sb[:, k * 4 + cc * 2 + oc, :],
                            tmul[k][cc][:],
                            start=(idx == 0),
                            stop=(idx == 17),
                        )
                        idx += 1
                nc.scalar.activation(
                    conv_sb[:, oc, t * TS: (t + 1) * TS], ps[:], AF.Copy,
                    accum_out=s1cols[:, oc, t: t + 1],
                )
                sq = sq_pool.tile([128, TS], BF16, tag="sq")
                nc.scalar.activation(
                    sq[:], ps[:], AF.Square,
                    accum_out=s2cols[:, oc, t: t + 1],
                )

        if dbg is not None:
            nc.gpsimd.dma_start(out=bass.AP(
                tensor=dbg["conv"].tensor, offset=0,
                ap=[[N, 128], [128 * N, 2], [1, N]],
            ), in_=conv_sb[:])

        # ---- phase 3: stats all-reduce + BN coefficients ----
        stats = persist.tile([128, 4], F32)
        nc.vector.tensor_reduce(stats[:, 0:2], s1cols[:], axis=mybir.AxisListType.X,
                                op=ALU.add)
        nc.vector.tensor_reduce(stats[:, 2:4], s2cols[:], axis=mybir.AxisListType.X,
                                op=ALU.add)
        cc_in = dram.tile([128, 4], F32)
        cc_out = dram.tile([128, 4], F32)
        nc.sync.dma_start(out=cc_in[:], in_=stats[:])
        if single:
            nc.sync.dma_start(out=cc_out[:], in_=cc_in[:])
        else:
            nc.gpsimd.collective_compute(
                "AllReduce", ALU.add,
                replica_groups=[list(range(NUM_CORES))],
                ins=[cc_in.opt()], outs=[cc_out.opt()],
            )
        red = persist.tile([128, 4], F32)
        nc.sync.dma_start(out=red[:], in_=cc_out[:])
        if dbg is not None:
            nc.sync.dma_start(out=dbg["stats"][:], in_=stats[:])
            nc.sync.dma_start(out=dbg["red"][:], in_=red[:])

        m = persist.tile([128, 2], F32)
        nc.vector.tensor_scalar_mul(m[:], red[:, 0:2], INV_COUNT)
        e2 = persist.tile([128, 2], F32)
        nc.vector.tensor_scalar_mul(e2[:], red[:, 2:4], INV_COUNT)
        var = persist.tile([128, 2], F32)
        nc.vector.tensor_mul(var[:], m[:], m[:])
        nc.vector.tensor_sub(var[:], e2[:], var[:])
        # rinv = 1/sqrt(var+eps) = exp(-0.5*ln(var+eps))
        lv = persist.tile([128, 2], F32)
        nc.scalar.activation(lv[:], var[:], AF.Ln, bias=bneps[:])
        rinv = persist.tile([128, 2], F32)
        nc.scalar.activation(rinv[:], lv[:], AF.Exp, scale=-0.5)
        scl = persist.tile([128, 2], F32)
        nc.vector.tensor_mul(scl[:], rinv[:], gam_sb[:])
        bia = persist.tile([128, 2], F32)
        nc.vector.tensor_mul(bia[:], m[:], scl[:])
        nc.vector.tensor_sub(bia[:], bet_sb[:], bia[:])

        # ---- phase 4: BN + ReLU + store (split across ScalarE and VectorE) ----
        fin_pool = ctx.enter_context(tc.tile_pool(name=f"fin{uid}", bufs=6))
        for t in range(NT):
            for oc in range(2):
                fin = fin_pool.tile([128, TS], F32, tag="fin")
                if (2 * t + oc) % 2 == 0:
                    nc.scalar.activation(
                        fin[:], conv_sb[:, oc, t * TS: (t + 1) * TS], AF.Relu,
                        bias=bia[:, oc: oc + 1], scale=scl[:, oc: oc + 1],
                    )
                else:
                    nc.vector.tensor_scalar(
                        out=fin[:], in0=conv_sb[:, oc, t * TS: (t + 1) * TS],
                        scalar1=scl[:, oc: oc + 1], scalar2=bia[:, oc: oc + 1],
                        op0=ALU.mult, op1=ALU.add,
                    )
                    nc.vector.tensor_scalar_max(fin[:], fin[:], 0.0)
                nc.sync.dma_start(
                    out=out[oc * 128: (oc + 1) * 128, t * TS: (t + 1) * TS],
                    in_=fin[:],
                )


def _prep_inputs(x, coords, rotations, distances, W, gamma, beta):
    """Host-side sharding/layout prep. Returns per-core input maps."""
    bf = ml_dtypes.bfloat16
    # weights: [o, c, k] -> 36 lhsT tiles [(k, cc, oc), c, o]
    wt = W.reshape(2, 128, 2, 128, K)            # [oc, o, cc, c, k]
    wt = wt.transpose(4, 2, 0, 3, 1)             # [k, cc, oc, c, o]
    wt = np.ascontiguousarray(wt.reshape(36, 128, 128), dtype=bf)
    gam2 = np.ascontiguousarray(gamma.reshape(2, 128), dtype=np.float32)
    bet2 = np.ascontiguousarray(beta.reshape(2, 128), dtype=np.float32)

    # gather index for the blocked-(tap,group) g layout
    idx = ((np.arange(K) * DIL)[:, None, None]
           + (np.arange(NGRP) * GRP)[None, :, None]
           + np.arange(GRP)[None, None, :])            # [K, NGRP, GRP]
    in_maps = []
    for b in range(NUM_CORES):
        xpad = np.zeros((CIN, NXP), dtype=bf)
        xpad[:, PAD: PAD + N] = x[b].astype(bf)
        cd4 = np.zeros((4, NGP), dtype=np.float32)
        cd4[:3, PAD: PAD + N] = coords[b]
        cd4[3, PAD: PAD + N] = distances[b]
        rot = np.zeros((3, NGP), dtype=np.float32)
        rot[:, PAD: PAD + N] = rotations[b]
        idx0 = idx - (np.arange(K) * DIL)[:, None, None] + PAD  # unshifted, k-replicated
        cds_h = cd4[:, idx].transpose(1, 2, 0, 3).reshape(PGK, 4, GRP)
        cd0_h = cd4[:, idx0].transpose(1, 2, 0, 3).reshape(PGK, 4, GRP)
        r_s = rot[:, idx].transpose(1, 2, 0, 3)        # [K, NGRP, 3, GRP]
        r_0 = rot[:, idx0].transpose(1, 2, 0, 3)
        ral_h = np.stack([r_0, r_s], axis=2).reshape(PGK, 2, 3, GRP)
        in_maps.append({
            "xp": np.ascontiguousarray(xpad.reshape(2, 128, NXP)),
            "cds": np.ascontiguousarray(cds_h),
            "cd0": np.ascontiguousarray(cd0_h),
            "ral": np.ascontiguousarray(ral_h),
            "wt": wt,
            "gam": gam2,
            "bet": bet2,
        })
    return in_maps


def kernel(x, coords, rotations, distances, W, bias, gamma, beta):
    if "nc" not in _CACHE:
        _CACHE["nc"] = _build_kernel()
    nc = _CACHE["nc"]
    in_maps = _prep_inputs(x, coords, rotations, distances, W, gamma, beta)
    res = run_bass_kernel_spmd(nc, in_maps, list(range(NUM_CORES)), trace=False)
    return np.stack([res.results[b]["out"] for b in range(NUM_CORES)], axis=0)
